# revision 44
# baseline (speedup 1.0000x reference)
"""Trainium2 Bass kernel for nn_DimeNetBlock (gnn_message_passing).

Algorithm notes (derived from the reference):
- compute_angle(v, -v) makes the coord path degenerate: for every non-self-loop
  edge the angle is arccos(clip(-(vn.vn))) ~= pi; pi*W1[16] is folded into the
  first-layer bias exactly (fp32) and only the tiny per-edge deviation
  (or -pi/2 for self-loops) travels with the edge features, so no coord
  gather is needed on device.
- The second edge-MLP linear (W2) commutes with the segment sum and is folded
  into the first update-MLP linear: agg@W2@W3 = (sum_e SiLU(h1_e)) @ (W2@W3).
- Edges are sorted by destination node; nodes are sharded across the 8 cores
  (12500 each) so no collective is needed. Within a core, nodes go to 4
  streams x degree-class grids (classes every 2, regions emitted largest-D
  first so node slots complete nearly in lockstep with the column stream at
  the end) with pad slots; a pad-flag input row with a -240 weight (max fp8
  exponent that hardware e4m3 still decodes as a number) makes SiLU(pad)==0,
  so the whole device pipeline is dense and index-free:
    matmul1 (block-diag W1, fp8 rhs+lhsT, K=128) -> SiLU (ScalarE, the
    pacing engine at ~1 elem/lane/cycle) -> pairwise-halving segment tree
    (VectorE, fp16) -> update MLP (block-diag) -> + x -> output.
- The 4 streams live in 4 blocks of 32 SBUF partitions (18 rows of real
  features each); two matmuls per column chunk (streams A|B and C|D) keep
  K=128 while M=128 caps output at 2 streams x 64 features.
- Phase A uses 1536-col PSUM chunks (2 bufs, 6 banks); the update MLP runs
  on its own 512-col PSUM ring (2 bufs, 2 banks) interleaved into phase A
  with two-group hysteresis and tails deferred one boundary, so the two
  phases never contend for a PSUM buffer and the tail is only the last
  node chunk. Inputs stream on the sync+gpsimd DMA queues (sync+scalar
  during ramp); outputs leave bf16 on sync so the gpsimd drain overlaps.
- Host gathers/transposes the per-core feature-major outputs at the end.
"""

import numpy as np
import ml_dtypes

try:
    import concourse.bass as bass  # noqa: F401
except Exception:  # pragma: no cover
    import sys

    sys.path.insert(0, "/opt/trn_rl_repo")
import concourse.tile as tile
from concourse import bacc, mybir
from concourse.bass_utils import run_bass_kernel_spmd

BF16 = ml_dtypes.bfloat16
FP8 = ml_dtypes.float8_e4m3fn

N_NODES = 100000
E_EDGES = 3200000
EMBED = 64
N_CORES = 8
NPC = N_NODES // N_CORES
N_STREAMS = 4
PAD_W = -240.0
SUBREGION_COLS = 7680
PSUM_CHUNK = 1536
MM_N = 512

TRACE = False  # set by test.py to capture a profile


# --------------------------------------------------------------------------
# host-side layout
# --------------------------------------------------------------------------

def _compute_angle(coord, row, col):
    v = (coord[row] - coord[col]).astype(np.float32)
    norm = np.maximum(
        np.sqrt((v * v).sum(-1, keepdims=True)).astype(np.float32), np.float32(1e-12)
    )
    vn = (v / norm).astype(np.float32)
    cos = (-(vn * vn).sum(-1)).astype(np.float32)
    lo = np.float32(-1.0 + 1e-8)
    hi = np.float32(1.0 - 1e-8)
    return np.arccos(np.clip(cos, lo, hi)).astype(np.float32)


def _make_layout(deg):
    """Degree classes every 4; per-core nodes are dealt degree-descending
    round-robin into 4 streams, class capacities are the minimal
    suffix-feasible values over all (core, stream) pairs, and each stream
    fills its class slots in degree order (nodes may be promoted to a
    larger class to fill capacity instead of creating dummy slots)."""
    dmax = int(deg.max())
    classes_D = list(range(8, dmax + 2, 2))
    while classes_D[-1] < dmax:
        classes_D.append(classes_D[-1] + 2)
    n_cls = len(classes_D)
    Darr = np.array(classes_D)

    cls_of_deg = np.searchsorted(Darr, np.arange(dmax + 1))
    node_cls = cls_of_deg[deg]

    # deal nodes (deg desc) round-robin into 4 streams per core
    per_cq_nodes = []  # [core][q] -> node array sorted by deg desc
    suffix_req = np.zeros(n_cls + 1, np.int64)
    for c in range(N_CORES):
        lo, hi = c * NPC, (c + 1) * NPC
        order = np.argsort(-deg[lo:hi], kind="stable") + lo
        qs = [order[q::N_STREAMS] for q in range(N_STREAMS)]
        per_cq_nodes.append(qs)
        for q in range(N_STREAMS):
            kcnt = np.bincount(node_cls[qs[q]], minlength=n_cls)
            sfx = np.cumsum(kcnt[::-1])[::-1]
            suffix_req[:n_cls] = np.maximum(suffix_req[:n_cls], sfx)

    # minimal capacities: M_k = max(0, R_k - sum_{j>k} M_j)
    M = [0] * n_cls
    tail = 0
    for k in range(n_cls - 1, -1, -1):
        M[k] = max(0, int(suffix_req[k]) - tail)
        tail += M[k]

    # per (core, stream): fill class slots from the largest class down with
    # the highest-degree remaining nodes
    per_core_cls_nodes = []
    for c in range(N_CORES):
        entry = [[None] * N_STREAMS for _ in range(n_cls)]
        for q in range(N_STREAMS):
            nodes = per_cq_nodes[c][q]  # deg desc
            idx = 0
            for k in range(n_cls - 1, -1, -1):
                take = nodes[idx : idx + M[k]]
                entry[k][q] = take
                idx += len(take)
            assert idx == len(nodes)
        per_core_cls_nodes.append([tuple(entry[k]) for k in range(n_cls)])

    regions = []
    col_off = 0
    node_off = 0
    for k in range(n_cls - 1, -1, -1):
        D = classes_D[k]
        if M[k] == 0:
            continue
        m_max = max(1, SUBREGION_COLS // D)
        left = M[k]
        while left > 0:
            m = min(m_max, left)
            regions.append((D, m, col_off, node_off))
            col_off += D * m
            node_off += m
            left -= m
    return {
        "classes_D": classes_D,
        "class_M": M,
        "regions": regions,
        "S": col_off,
        "M_tot": node_off,
        "per_core_cls_nodes": per_core_cls_nodes,
    }


def _build_host_arrays(x, coord, rbf_feature, edge_index, W1, b1, b4, layout):
    row = np.ascontiguousarray(edge_index[0]).astype(np.int64)
    col = np.ascontiguousarray(edge_index[1]).astype(np.int64)
    angle = _compute_angle(coord, row, col)

    regions = layout["regions"]
    pccn = layout["per_core_cls_nodes"]
    S, M_tot = layout["S"], layout["M_tot"]
    classes_D = layout["classes_D"]
    n_cls = len(classes_D)

    cls_regions = [[] for _ in range(n_cls)]
    for D, m, co, no in regions:
        cls_regions[classes_D.index(D)].append((m, co, no))

    node_stream = np.full(N_NODES, -1, np.int8)
    node_colbase = np.zeros(N_NODES, np.int64)
    node_m = np.zeros(N_NODES, np.int64)
    node_li = np.zeros(N_NODES, np.int64)
    node_pos = np.zeros(N_NODES, np.int64)
    for c in range(N_CORES):
        for k in range(n_cls):
            for q in range(N_STREAMS):
                nodes = pccn[c][k][q]
                if len(nodes) == 0:
                    continue
                idx = 0
                for m, co, no in cls_regions[k]:
                    take = nodes[idx : idx + m]
                    if len(take) == 0:
                        break
                    li = np.arange(len(take))
                    node_stream[take] = q
                    node_colbase[take] = co
                    node_m[take] = m
                    node_li[take] = li
                    node_pos[take] = no + li
                    idx += m

    order = np.argsort(row, kind="stable")
    row_s = row[order]
    deg = np.bincount(row_s, minlength=N_NODES)
    starts = np.zeros(N_NODES + 1, np.int64)
    np.cumsum(deg, out=starts[1:])
    pos_in_seg = np.arange(E_EDGES) - starts[row_s]
    ecol = node_colbase[row_s] + pos_in_seg * node_m[row_s] + node_li[row_s]
    equarter = node_stream[row_s]
    ecore = row_s // NPC

    rbfT_s = np.ascontiguousarray(rbf_feature.T).astype(FP8)[:, order]
    # angle deviation from pi (pi*W1[16] is folded into bias1 exactly); the
    # deviation is ~5e-4 for normal edges and -pi/2 for self-loops, so fp8
    # rounding of it is negligible
    angle_s = (angle[order].astype(np.float64) - np.pi).astype(np.float32).astype(FP8)

    rbf_devs, xw_devs = [], []
    for c in range(N_CORES):
        dev = np.zeros((128, S), dtype=FP8)
        for q in range(N_STREAMS):
            dev[32 * q + 17, :] = FP8(1.0)  # pad flag
            sel = (ecore == c) & (equarter == q)
            cols = ecol[sel]
            r0 = 32 * q
            dev[r0 : r0 + 16, cols] = rbfT_s[:, sel]
            dev[r0 + 16, cols] = angle_s[sel]
            dev[r0 + 17, cols] = FP8(0.0)
        rbf_devs.append(dev)

        # xw: [128, 2*M_tot] -- cols [0,M_tot) streams A|B, [M_tot,2M) C|D
        xw = np.zeros((128, 2 * M_tot), np.float32)
        nodes = np.arange(c * NPC, (c + 1) * NPC)
        xb = (x[nodes].astype(np.float32) + b4[None, :].astype(np.float32)).T
        st = node_stream[nodes]
        ps = node_pos[nodes]
        for q in range(N_STREAMS):
            rows = slice(0, 64) if q % 2 == 0 else slice(64, 128)
            off = 0 if q < 2 else M_tot
            xw[rows, off + ps[st == q]] = xb[:, st == q]
        xw_devs.append(xw.astype(BF16))

    blk = np.concatenate(
        [
            W1[:16].astype(np.float32),
            W1[16:17].astype(np.float32),
            np.full((1, EMBED), PAD_W, np.float32),
        ],
        axis=0,
    )  # [18, 64]
    lhsT_E = np.zeros((128, 128), np.float32)
    lhsT_E[0:18, 0:64] = blk  # stream A (rows 0-17) -> out cols 0-63
    lhsT_E[32:50, 64:128] = blk  # stream B -> out cols 64-127
    lhsT_O = np.zeros((128, 128), np.float32)
    lhsT_O[64:82, 0:64] = blk  # stream C
    lhsT_O[96:114, 64:128] = blk  # stream D

    meta = {"node_stream": node_stream, "node_pos": node_pos, "deg": deg}
    return rbf_devs, xw_devs, lhsT_E.astype(FP8), lhsT_O.astype(FP8), meta


def _blockdiag2(W):
    out = np.zeros((128, 128), np.float32)
    out[0:64, 0:64] = W
    out[64:128, 64:128] = W
    return out


# --------------------------------------------------------------------------
# device kernel
# --------------------------------------------------------------------------

def _emit_tree(nc, s_sb, sum_bf, D, m, no, f16_add):
    """Pairwise-halving segment reduction along the slot axis, in place."""
    dd = D
    while dd > 1:
        if dd % 2 == 1:
            f16_add(s_sb[:, :m], s_sb[:, :m], s_sb[:, (dd - 1) * m : dd * m])
            dd -= 1
        elif dd == 2:
            f16_add(sum_bf[:, no : no + m], s_sb[:, :m], s_sb[:, m : 2 * m])
            dd = 1
        else:
            half = (dd // 2) * m
            f16_add(s_sb[:, :half], s_sb[:, :half], s_sb[:, half : 2 * half])
            dd //= 2
    if D == 1:
        f16_add(sum_bf[:, no : no + m], s_sb[:, :m], s_sb[:, :m])  # unreachable


def _build_nc(layout, b2W3_nonzero):
    S, M_tot = layout["S"], layout["M_tot"]
    regions = layout["regions"]
    f16 = mybir.dt.float16
    bf16 = mybir.dt.bfloat16
    f32 = mybir.dt.float32
    f8 = mybir.dt.float8e4
    SILU = mybir.ActivationFunctionType.Silu

    nc = bacc.Bacc("TRN2", target_bir_lowering=False, debug=False, num_devices=N_CORES)
    rbf_d = nc.dram_tensor("rbf_dev", [128, S], f8, kind="ExternalInput")
    xw_d = nc.dram_tensor("xw_dev", [128, 2 * M_tot], bf16, kind="ExternalInput")
    lhsE_d = nc.dram_tensor("lhsT_E", [128, 128], f8, kind="ExternalInput")
    lhsO_d = nc.dram_tensor("lhsT_O", [128, 128], f8, kind="ExternalInput")
    bias1_d = nc.dram_tensor("bias1", [128, 1], f32, kind="ExternalInput")
    wup_d = nc.dram_tensor("Wup", [128, 128], bf16, kind="ExternalInput")
    bias3_d = nc.dram_tensor("bias3", [128, 1], f32, kind="ExternalInput")
    w4_d = nc.dram_tensor("W4x2", [128, 128], bf16, kind="ExternalInput")
    if b2W3_nonzero:
        degb_d = nc.dram_tensor("deg_dev", [4, 2 * M_tot], bf16, kind="ExternalInput")
        b2w3_d = nc.dram_tensor("b2w3cat", [2, 128], bf16, kind="ExternalInput")
    out_d = nc.dram_tensor("out_dev", [128, 2 * M_tot], bf16, kind="ExternalOutput")

    with tile.TileContext(nc) as tc:
        with (
            tc.tile_pool(name="const", bufs=1) as cpool,
            tc.tile_pool(name="rbf", bufs=6) as rbf_pool,
            tc.tile_pool(name="s", bufs=3) as s_pool,
            tc.tile_pool(name="psum", bufs=2, space="PSUM") as psum_pool,
            tc.tile_pool(name="psumB", bufs=2, space="PSUM") as psumB_pool,
            tc.tile_pool(name="b", bufs=4) as b_pool,
        ):
            # only the tensors needed by phase A are loaded up front; the
            # phase-B constants are DMA'd after phase-A emission so they do
            # not clog the queue ahead of the first rbf region.
            # tiny dummy activation up front: forces the Silu table load
            # to overlap the input DMAs instead of gating the first real ACT
            warm = cpool.tile([1, 8], f32)
            nc.gpsimd.memset(warm[:], 0.0)
            nc.scalar.activation(
                warm[:], warm[:], mybir.ActivationFunctionType.Silu
            )
            bias1 = cpool.tile([128, 1], f32)
            nc.sync.dma_start(out=bias1[:], in_=bias1_d[:, :])
            lhsE = cpool.tile([128, 128], f8)
            nc.sync.dma_start(out=lhsE[:], in_=lhsE_d[:, :])
            lhsO = cpool.tile([128, 128], f8)
            nc.sync.dma_start(out=lhsO[:], in_=lhsO_d[:, :])
            sum_AB = cpool.tile([128, M_tot], bf16)
            sum_CD = cpool.tile([128, M_tot], bf16)

            def f16_add(out, a, b):
                nc.vector.tensor_add(out=out, in0=a, in1=b)

            def emit_b_chunk(half, t0, w):
                sum_t = sum_AB if half == 0 else sum_CD
                base = half * M_tot
                ps3 = psumB_pool.tile([128, w], f32, tag="psB")
                for m0 in range(0, w, MM_N):
                    mw = min(MM_N, w - m0)
                    nc.tensor.matmul(
                        out=ps3[:, m0 : m0 + mw],
                        lhsT=wup[:],
                        rhs=sum_t[:, t0 + m0 : t0 + m0 + mw],
                        start=True,
                        stop=not b2W3_nonzero,
                    )
                    if b2W3_nonzero:
                        nc.tensor.matmul(
                            out=ps3[:, m0 : m0 + mw],
                            lhsT=b2w3[0:1, :],
                            rhs=degb[
                                2 * half : 2 * half + 1,
                                base + t0 + m0 : base + t0 + m0 + mw,
                            ],
                            start=False,
                            stop=False,
                        )
                        nc.tensor.matmul(
                            out=ps3[:, m0 : m0 + mw],
                            lhsT=b2w3[1:2, :],
                            rhs=degb[
                                2 * half + 1 : 2 * half + 2,
                                base + t0 + m0 : base + t0 + m0 + mw,
                            ],
                            start=False,
                            stop=True,
                        )
                s3 = b_pool.tile([128, w], bf16, tag="s3")
                nc.scalar.activation(s3[:, :w], ps3[:, :w], SILU, bias=bias3[:])
                return s3

            def emit_b_tail(half, t0, w, s3):
                base = half * M_tot
                ps4 = psumB_pool.tile([128, w], f32, tag="psB")
                for m0 in range(0, w, MM_N):
                    mw = min(MM_N, w - m0)
                    nc.tensor.matmul(
                        out=ps4[:, m0 : m0 + mw],
                        lhsT=w4[:],
                        rhs=s3[:, m0 : m0 + mw],
                        start=True,
                        stop=True,
                    )
                ob = b_pool.tile([128, w], bf16, tag="ob")
                nc.vector.tensor_add(
                    out=ob[:, :w],
                    in0=ps4[:, :w],
                    in1=xw[:, base + t0 : base + t0 + w],
                )
                nc.sync.dma_start(
                    out=out_d[:, base + t0 : base + t0 + w], in_=ob[:, :w]
                )

            # ---- phase A: edge MLP + segment-sum tree, with phase-B
            # update-MLP chunks interleaved as their node slots complete ----
            # group contiguous regions into shared load tiles (fewer DMAs:
            # the tiny class-remainder regions ride along with neighbors);
            # group caps grow so the pipeline ramps quickly
            groups = []
            cur, curF = [], 0
            cap_seq = [256, 1536, 3072, 4608]
            for reg in regions:
                F = reg[0] * reg[1]
                cap = cap_seq[len(groups)] if len(groups) < len(cap_seq) else SUBREGION_COLS
                if curF + F > cap and cur:
                    groups.append((cur, curF))
                    cur, curF = [], 0
                cur.append(reg)
                curF += F
            if cur:
                groups.append((cur, curF))

            # phase-B chunk schedule: 1024-col chunks per half, emitted with
            # one-group hysteresis after their node slots' trees are done
            b_bounds = list(range(0, M_tot, 512)) + [M_tot]
            # one (boundary, half) per entry: at most one B head is inserted
            # per group, halving the PE work spike at each emission point
            b_sched = [
                (b_bounds[i], b_bounds[i + 1] - b_bounds[i], half)
                for i in range(len(b_bounds) - 1)
                for half in (0, 1)
            ]
            b_state = {"next": 0}
            pend = []

            def maybe_emit_b(node_done):
                # tails from earlier boundaries first (inputs long ready, so
                # they insert zero-wait work into the PE/DVE queues)
                while pend and pend[0][4] < b_state["next"]:
                    h, t, w, s3, _ = pend.pop(0)
                    emit_b_tail(h, t, w, s3)
                emitted = 0
                while b_state["next"] < len(b_sched):
                    t0, w, half = b_sched[b_state["next"]]
                    if node_done < t0 + w:
                        break
                    pend.append(
                        (half, t0, w, emit_b_chunk(half, t0, w), b_state["next"])
                    )
                    b_state["next"] += 1
                    emitted += 1
                    if node_done >= M_tot:
                        # final drain: keep heads two ahead of tails so the
                        # w4+add+dma pipeline stays full instead of running
                        # all heads then all tails serially
                        while len(pend) > 2:
                            h, t, ww, s3, _ = pend.pop(0)
                            emit_b_tail(h, t, ww, s3)
                    else:
                        break  # at most one new head per group boundary

            dma_q = [nc.sync, nc.gpsimd]
            out_q = [0]
            qi = 0
            done_after = []
            n_xw = 4  # xw arrives in quarters, interleaved with the rbf stream
            xw_step = -(-2 * M_tot // n_xw)
            for gi, (grp, Fg) in enumerate(groups):
                if gi >= 2:
                    maybe_emit_b(done_after[gi - 2])
                co0 = grp[0][2]
                rbf_sb = rbf_pool.tile([128, Fg], f8, tag="rbf")
                # finer transfers during the ramp halve the arrival jitter
                # the first activations see; a dma_start costs its issuing
                # engine ~0.7us, so the scalar queue is never used (that
                # would stall the pacing engine between early activates)
                step = PSUM_CHUNK if gi >= 4 else PSUM_CHUNK // 2
                for d0 in range(0, Fg, step):
                    dw = min(step, Fg - d0)
                    q = dma_q[qi % 2]
                    q.dma_start(
                        out=rbf_sb[:, d0 : d0 + dw],
                        in_=rbf_d[:, co0 + d0 : co0 + d0 + dw],
                    )
                    qi += 1
                if gi == 0:
                    # tile created now (closures bind it); DMAs deferred so
                    # the gpsimd queue is all-rbf during the ramp
                    xw = cpool.tile([128, 2 * M_tot], bf16)
                elif gi == 3:
                    # phase-B constants: not needed until the first B head
                    # (~group 6), so they stay off the ramp-critical queues
                    wup = cpool.tile([128, 128], bf16)
                    nc.gpsimd.dma_start(out=wup[:], in_=wup_d[:, :])
                    bias3 = cpool.tile([128, 1], f32)
                    nc.gpsimd.dma_start(out=bias3[:], in_=bias3_d[:, :])
                    w4 = cpool.tile([128, 128], bf16)
                    nc.gpsimd.dma_start(out=w4[:], in_=w4_d[:, :])
                    if b2W3_nonzero:
                        degb = cpool.tile([4, 2 * M_tot], bf16)
                        nc.gpsimd.dma_start(out=degb[:], in_=degb_d[:, :])
                        b2w3 = cpool.tile([2, 128], bf16)
                        nc.gpsimd.dma_start(out=b2w3[:], in_=b2w3_d[:, :])
                elif 4 <= gi <= 3 + n_xw:
                    x0 = (gi - 4) * xw_step
                    x1 = min(2 * M_tot, x0 + xw_step)
                    dma_q[qi % 2].dma_start(
                        out=xw[:, x0:x1], in_=xw_d[:, x0:x1]
                    )
                    qi += 1
                for lhs, s_tag, sum_t in (
                    (lhsE, "sAB", sum_AB),
                    (lhsO, "sCD", sum_CD),
                ):
                    s_sb = s_pool.tile([128, Fg], f16, tag=s_tag)
                    for c0 in range(0, Fg, PSUM_CHUNK):
                        cw = min(PSUM_CHUNK, Fg - c0)
                        ps = psum_pool.tile([128, cw], f32, tag="ps")
                        for m0 in range(0, cw, MM_N):
                            mw = min(MM_N, cw - m0)
                            nc.tensor.matmul(
                                out=ps[:, m0 : m0 + mw],
                                lhsT=lhs[:],
                                rhs=rbf_sb[:, c0 + m0 : c0 + m0 + mw],
                                start=True,
                                stop=True,
                            )
                        nc.scalar.activation(
                            s_sb[:, c0 : c0 + cw], ps[:, :cw], SILU, bias=bias1[:]
                        )
                    for D, m, co, no in grp:
                        _emit_tree(
                            nc, s_sb[:, co - co0 : co - co0 + D * m], sum_t,
                            D, m, no, f16_add,
                        )
                done_after.append(max(no + m for D, m, co, no in grp))

            maybe_emit_b(M_tot)
            for h, t, w, s3, _ in pend:
                emit_b_tail(h, t, w, s3)

    nc.compile()
    return nc


# --------------------------------------------------------------------------
# entry point
# --------------------------------------------------------------------------

_LAST_RESULTS = {}


def prepare(x, coord, rbf_feature, edge_index, W1, b1, W2, b2, W3, b3, W4, b4):
    """Host prep + NEFF build. Returns (nc, in_maps, meta, layout)."""
    x = np.asarray(x, np.float32)
    coord = np.asarray(coord, np.float32)
    rbf_feature = np.asarray(rbf_feature, np.float32)
    edge_index = np.asarray(edge_index)

    row = np.ascontiguousarray(edge_index[0]).astype(np.int64)
    deg = np.bincount(row, minlength=N_NODES)
    layout = _make_layout(deg)

    rbf_devs, xw_devs, lhsT_E, lhsT_O, meta = _build_host_arrays(
        x, coord, rbf_feature, edge_index, W1, b1, b4, layout
    )

    W23 = (W2.astype(np.float64) @ W3.astype(np.float64)).astype(np.float32)
    b2W3 = (b2.astype(np.float64) @ W3.astype(np.float64)).astype(np.float32)
    b2nz = bool(np.any(b2W3 != 0.0))

    wup = _blockdiag2(W23).astype(BF16)
    w4x2 = _blockdiag2(W4.astype(np.float32)).astype(BF16)
    b1a = (b1.astype(np.float64) + np.pi * W1[16].astype(np.float64)).astype(np.float32)
    bias1 = np.concatenate([b1a, b1a]).astype(np.float32)[:, None]
    bias3 = np.concatenate([b3, b3]).astype(np.float32)[:, None]

    nc = _build_nc(layout, b2nz)

    in_maps = []
    for c in range(N_CORES):
        im = {
            "rbf_dev": np.ascontiguousarray(rbf_devs[c]),
            "xw_dev": np.ascontiguousarray(xw_devs[c]),
            "lhsT_E": lhsT_E,
            "lhsT_O": lhsT_O,
            "bias1": bias1,
            "Wup": wup,
            "bias3": bias3,
            "W4x2": w4x2,
        }
        if b2nz:
            M_tot = layout["M_tot"]
            degb = np.zeros((4, 2 * M_tot), np.float32)
            nodes = np.arange(c * NPC, (c + 1) * NPC)
            st = meta["node_stream"][nodes]
            ps = meta["node_pos"][nodes]
            dg = meta["deg"][nodes].astype(np.float32)
            for q in range(N_STREAMS):
                r = q % 2
                off = 0 if q < 2 else M_tot
                degb[2 * (q // 2) + r, off + ps[st == q]] = dg[st == q]
            im["deg_dev"] = degb.astype(BF16)
            b2w3cat = np.zeros((2, 128), np.float32)
            b2w3cat[0, 0:64] = b2W3
            b2w3cat[1, 64:128] = b2W3
            im["b2w3cat"] = b2w3cat.astype(BF16)
        in_maps.append(im)
    return nc, in_maps, meta, layout


def postprocess(results, meta, layout):
    M_tot = layout["M_tot"]
    out = np.empty((N_NODES, EMBED), np.float32)
    ns, ps = meta["node_stream"], meta["node_pos"]
    for c in range(N_CORES):
        arr = results[c]["out_dev"]
        nodes = np.arange(c * NPC, (c + 1) * NPC)
        st = ns[nodes]
        pp = ps[nodes]
        for q in range(N_STREAMS):
            rows = slice(0, 64) if q % 2 == 0 else slice(64, 128)
            off = 0 if q < 2 else M_tot
            sel = st == q
            out[nodes[sel]] = arr[rows, off + pp[sel]].T
    return out


def kernel(x, coord, rbf_feature, edge_index, W1, b1, W2, b2, W3, b3, W4, b4):
    nc, in_maps, meta, layout = prepare(
        x, coord, rbf_feature, edge_index, W1, b1, W2, b2, W3, b3, W4, b4
    )
    res = run_bass_kernel_spmd(nc, in_maps, core_ids=list(range(N_CORES)), trace=TRACE)
    _LAST_RESULTS["res"] = res
    return postprocess(res.results, meta, layout)

